# revision 1
# baseline (speedup 1.0000x reference)
"""Distributed attention layer kernel for 8 TRN2 NeuronCores.

Reference computation (f32):
    Q = q @ W_q; K = k @ W_k; V = v @ W_v
    out = softmax((Q @ K^T)/sqrt(d_k)) @ V

Sharding: rows of q/k/v are split 8 ways (sequence parallel). Each core
projects its own shards, the K^T/V projections are all-gathered (bf16),
and each core computes its 512-row slice of the attention output.

Precision: the score path (projections of q/k and Q@K^T) uses a 3-term
compensated fp16 split (x = hi + lo, products hi*hi + hi*lo + lo*hi,
f32 PSUM accumulation) giving ~1e-7 relative score accuracy at 16-bit
matmul throughput. The V path runs in plain fp16 (~5e-4 relative).
Softmax itself is f32 (ACT exp with per-row max bias, fused row-sum).
Measured end-to-end error vs the f32 reference: ~4e-4.
"""

import os
import sys

for _p in ("/opt/pypackages", "/opt/trn_rl_repo"):
    if _p not in sys.path:
        sys.path.insert(0, _p)

import numpy as np

N_Q, N_KV, DIM = 4096, 4096, 1024  # D_K = D_V = DIM (square weights)
CORES = 8

P = 128


def build_attention(nq=N_Q, dim=DIM, cores=CORES):
    """Build the per-core Bass graph (SPMD; identical on all cores)."""
    import concourse.bass as bass
    import concourse.mybir as mybir
    from concourse import bacc
    from concourse.masks import make_identity
    from concourse.tile import TileContext

    dt = mybir.dt
    f32, bf16 = dt.float32, dt.float16  # "bf16" vars are fp16 now

    sh = nq // cores          # rows per core (512)
    n_ct = dim // P           # contraction tiles for projections (8)
    n_dt = dim // P           # d tiles (8)
    n_it = sh // P            # query-row tiles per core (4)
    n_jjt = sh // P           # kv-row tiles per core (4)
    n_eh = dim // 512         # 512-wide output column halves (2)
    EH = 512 if dim >= 512 else dim
    n_eh = max(1, dim // EH)
    n_jt = nq // P            # total kv j tiles (32)
    JG = 4                    # j-tiles per PV V-chunk
    n_jg = n_jt // JG         # V chunk count (8)
    IT_GROUP = 2              # i-tiles per PV psum group
    scale = 1.0 / float(np.sqrt(dim))

    nc = bacc.Bacc(num_devices=cores)

    # --- external I/O (per core: row shards of q/k/v, full weights) ---
    q_ext = nc.declare_dram_parameter("q", [sh, dim], f32, isOutput=False)
    k_ext = nc.declare_dram_parameter("k", [sh, dim], f32, isOutput=False)
    v_ext = nc.declare_dram_parameter("v", [sh, dim], f32, isOutput=False)
    wq_ext = nc.declare_dram_parameter("W_q", [dim, dim], f32, isOutput=False)
    wk_ext = nc.declare_dram_parameter("W_k", [dim, dim], f32, isOutput=False)
    wv_ext = nc.declare_dram_parameter("W_v", [dim, dim], f32, isOutput=False)
    out_ext = nc.declare_dram_parameter("out", [sh, dim], f32, isOutput=True)

    # --- internal DRAM for collectives ---
    hd = dim // 2
    bounce_khi1 = nc.dram_tensor("bounce_khi1", [hd, sh], bf16)
    bounce_khi2 = nc.dram_tensor("bounce_khi2", [hd, sh], bf16)
    bounce_klo1 = nc.dram_tensor("bounce_klo1", [hd, sh], bf16)
    bounce_klo2 = nc.dram_tensor("bounce_klo2", [hd, sh], bf16)
    bounce_v = nc.dram_tensor("bounce_v", [sh, dim], bf16)
    gath_khi1 = nc.dram_tensor("gath_khi1", [cores * hd, sh], bf16, addr_space="Shared")
    gath_khi2 = nc.dram_tensor("gath_khi2", [cores * hd, sh], bf16, addr_space="Shared")
    gath_klo1 = nc.dram_tensor("gath_klo1", [cores * hd, sh], bf16, addr_space="Shared")
    gath_klo2 = nc.dram_tensor("gath_klo2", [cores * hd, sh], bf16, addr_space="Shared")
    gath_v = nc.dram_tensor("gath_v", [cores * sh, dim], bf16, addr_space="Shared")

    rg = [list(range(cores))]

    with TileContext(nc) as tc:
        with (
            tc.tile_pool(name="const", bufs=1) as constp,
            tc.tile_pool(name="qt", bufs=1) as qtp,
            tc.tile_pool(name="stats", bufs=1) as statp,
        ):
            ident_f = constp.tile([P, P], f32, tag="idf", name="idf")
            make_identity(nc, ident_f)
            ident_b = constp.tile([P, P], bf16, tag="idb", name="idb")
            make_identity(nc, ident_b)

            qthi = qtp.tile([P, n_dt, sh], bf16, tag="qthi", name="qthi")
            qtlo = qtp.tile([P, n_dt, sh], bf16, tag="qtlo", name="qtlo")

            def load_split_w(w_ext, wpool, wstage, want_lo=True):
                """Load a [dim, dim] f32 weight, split into 16-bit hi/lo in
                SBUF laid out [c_in=128, ct, d]."""
                whi = wpool.tile([P, n_ct, dim], bf16, tag="whi", name="whi")
                wlo = wpool.tile([P, n_ct, dim], bf16, tag="wlo", name="wlo") if want_lo else None
                wsrc = w_ext.rearrange("(ct p) d -> p ct d", p=P)
                for ct in range(n_ct):
                    stg = wstage.tile([P, dim], f32, tag="wstg", name="wstg")
                    nc.sync.dma_start(stg[:], wsrc[:, ct])
                    nc.scalar.copy(whi[:, ct], stg[:])
                    if want_lo:
                        nc.vector.tensor_sub(wlo[:, ct], stg[:], whi[:, ct])
                return whi, wlo

            def load_transpose_split(x_ext, tpool, iost, tpsum, want_lo=True):
                """Load a [sh, dim] f32 input, transpose on PE, split to
                16-bit hi/lo in SBUF laid out [c_in=128, ct, row]."""
                xthi = tpool.tile([P, n_ct, sh], bf16, tag="thi", name="thi")
                xtlo = tpool.tile([P, n_ct, sh], bf16, tag="tlo", name="tlo") if want_lo else None
                xsrc = x_ext.rearrange("(it p) c -> p it c", p=P)
                for it in range(sh // P):
                    stg = iost.tile([P, dim], f32, tag="iostg", name="iostg")
                    nc.sync.dma_start(stg[:], xsrc[:, it])
                    for ct in range(n_ct):
                        ps = tpsum.tile([P, P], f32, tag="tps", name="tps")
                        nc.tensor.transpose(ps[:], stg[:, ct * P:(ct + 1) * P], ident_f)
                        dst = slice(it * P, (it + 1) * P)
                        nc.vector.tensor_copy(xthi[:, ct, dst], ps[:])
                        if want_lo:
                            nc.vector.tensor_sub(xtlo[:, ct, dst], ps[:], xthi[:, ct, dst])
                return xthi, xtlo

            with (
                tc.tile_pool(name="wstage", bufs=2) as wstage,
                tc.tile_pool(name="w", bufs=2) as wpool,
                tc.tile_pool(name="iost", bufs=2) as iost,
                tc.tile_pool(name="tin", bufs=2) as tpool,
                tc.tile_pool(name="kvout", bufs=2) as kvout,
                tc.tile_pool(name="tpsum", bufs=2, space="PSUM") as tpsum,
                tc.tile_pool(name="ppsum", bufs=2, space="PSUM") as ppsum,
            ):
                # ---- K path: project K^T shard (split-3), bounce out, AG ----
                wkhi, wklo = load_split_w(wk_ext, wpool, wstage)
                kthi, ktlo = load_transpose_split(k_ext, tpool, iost, tpsum)

                kt_hi_loc = kvout.tile([P, n_dt, sh], bf16, tag="kthi_loc", name="kthi_loc")
                kt_lo_loc = kvout.tile([P, n_dt, sh], bf16, tag="ktlo_loc", name="ktlo_loc")
                for dtt in range(n_dt):
                    ps = ppsum.tile([P, sh], f32, tag="pps", name="pps")
                    dsl = slice(dtt * P, (dtt + 1) * P)
                    n_acc = 3 * n_ct
                    i_acc = 0
                    for ct in range(n_ct):
                        for lhsT, rhs in (
                            (wkhi[:, ct, dsl], kthi[:, ct]),
                            (wkhi[:, ct, dsl], ktlo[:, ct]),
                            (wklo[:, ct, dsl], kthi[:, ct]),
                        ):
                            nc.tensor.matmul(
                                ps[:], lhsT, rhs,
                                start=(i_acc == 0), stop=(i_acc == n_acc - 1),
                            )
                            i_acc += 1
                    nc.scalar.copy(kt_hi_loc[:, dtt], ps[:])
                    nc.vector.tensor_sub(kt_lo_loc[:, dtt], ps[:], kt_hi_loc[:, dtt])

                nh = n_dt // 2
                bkh1 = bounce_khi1.rearrange("(dtt p) jj -> p dtt jj", p=P)
                bkh2 = bounce_khi2.rearrange("(dtt p) jj -> p dtt jj", p=P)
                bkl1 = bounce_klo1.rearrange("(dtt p) jj -> p dtt jj", p=P)
                bkl2 = bounce_klo2.rearrange("(dtt p) jj -> p dtt jj", p=P)
                nc.sync.dma_start(bkh1[:], kt_hi_loc[:, :nh])
                nc.gpsimd.collective_compute(
                    "AllGather", mybir.AluOpType.bypass, replica_groups=rg,
                    ins=[bounce_khi1.ap().opt()], outs=[gath_khi1.ap().opt()],
                )
                nc.sync.dma_start(bkh2[:], kt_hi_loc[:, nh:])
                nc.gpsimd.collective_compute(
                    "AllGather", mybir.AluOpType.bypass, replica_groups=rg,
                    ins=[bounce_khi2.ap().opt()], outs=[gath_khi2.ap().opt()],
                )
                nc.sync.dma_start(bkl1[:], kt_lo_loc[:, :nh])
                nc.gpsimd.collective_compute(
                    "AllGather", mybir.AluOpType.bypass, replica_groups=rg,
                    ins=[bounce_klo1.ap().opt()], outs=[gath_klo1.ap().opt()],
                )
                nc.sync.dma_start(bkl2[:], kt_lo_loc[:, nh:])
                nc.gpsimd.collective_compute(
                    "AllGather", mybir.AluOpType.bypass, replica_groups=rg,
                    ins=[bounce_klo2.ap().opt()], outs=[gath_klo2.ap().opt()],
                )

                # ---- Q path (local only): project Q^T (split-3) ----
                wqhi, wqlo = load_split_w(wq_ext, wpool, wstage)
                qth, qtl = load_transpose_split(q_ext, tpool, iost, tpsum)

                for dtt in range(n_dt):
                    ps = ppsum.tile([P, sh], f32, tag="pps", name="pps")
                    dsl = slice(dtt * P, (dtt + 1) * P)
                    n_acc = 3 * n_ct
                    i_acc = 0
                    for ct in range(n_ct):
                        for lhsT, rhs in (
                            (wqhi[:, ct, dsl], qth[:, ct]),
                            (wqhi[:, ct, dsl], qtl[:, ct]),
                            (wqlo[:, ct, dsl], qth[:, ct]),
                        ):
                            nc.tensor.matmul(
                                ps[:], lhsT, rhs,
                                start=(i_acc == 0), stop=(i_acc == n_acc - 1),
                            )
                            i_acc += 1
                    nc.scalar.copy(qthi[:, dtt], ps[:])
                    nc.vector.tensor_sub(qtlo[:, dtt], ps[:], qthi[:, dtt])

                # ---- V path: project V shard (plain 16-bit), bounce, AG ----
                wvh, _ = load_split_w(wv_ext, wpool, wstage, want_lo=False)
                vth, _ = load_transpose_split(v_ext, tpool, iost, tpsum, want_lo=False)

                v_loc = kvout.tile([P, n_jjt, dim], bf16, tag="v_loc", name="v_loc")
                for jjt in range(n_jjt):
                    jsl = slice(jjt * P, (jjt + 1) * P)
                    for eh in range(n_eh):
                        ps = ppsum.tile([P, EH], f32, tag="ppsv", name="ppsv")
                        esl = slice(eh * EH, (eh + 1) * EH)
                        for ct in range(n_ct):
                            nc.tensor.matmul(
                                ps[:], vth[:, ct, jsl], wvh[:, ct, esl],
                                start=(ct == 0), stop=(ct == n_ct - 1),
                            )
                        nc.scalar.copy(v_loc[:, jjt, esl], ps[:])

                bv = bounce_v.rearrange("(jjt p) e -> p jjt e", p=P)
                nc.sync.dma_start(bv[:], v_loc[:])

            # ================= attention phase =================
            m_t = [statp.tile([P, 1], f32, tag=f"m{it}", name=f"m{it}") for it in range(n_it)]
            tmpmax = statp.tile([P, 1], f32, tag="tmpmax", name="tmpmax")
            bias_t = [statp.tile([P, 1], f32, tag=f"b{it}", name=f"b{it}") for it in range(n_it)]
            ell_t = [statp.tile([P, 1], f32, tag=f"l{it}", name=f"l{it}") for it in range(n_it)]
            rl_t = [statp.tile([P, 1], f32, tag=f"r{it}", name=f"r{it}") for it in range(n_it)]

            gkh1 = gath_khi1.rearrange("(r dtt p) jj -> r p dtt jj", r=cores, p=P)
            gkh2 = gath_khi2.rearrange("(r dtt p) jj -> r p dtt jj", r=cores, p=P)
            gkl1 = gath_klo1.rearrange("(r dtt p) jj -> r p dtt jj", r=cores, p=P)
            gkl2 = gath_klo2.rearrange("(r dtt p) jj -> r p dtt jj", r=cores, p=P)
            gv = gath_v.rearrange("(jg jj p) e -> jg p jj e", jj=JG, p=P)

            with (
                tc.tile_pool(name="schunk", bufs=3) as schunk,
                tc.tile_pool(name="srow", bufs=n_it) as srow,
                tc.tile_pool(name="prow", bufs=n_it) as prow,
                tc.tile_pool(name="ptp", bufs=2) as ptp,
                tc.tile_pool(name="vchunk", bufs=2) as vchunk,
                tc.tile_pool(name="opool", bufs=2) as opool,
            ):
                s_sb = [srow.tile([P, nq], f32, tag="s", name="s") for _ in range(n_it)]

                # ---- scores: S[it, rr-chunk] with running row max ----
                _spsum_cm = tc.tile_pool(name="spsum", bufs=6, space="PSUM")
                spsum = _spsum_cm.__enter__()
                for rr in range(cores):
                    khi = schunk.tile([P, n_dt, sh], bf16, tag="khi", name="khi")
                    klo = schunk.tile([P, n_dt, sh], bf16, tag="klo", name="klo")
                    nc.sync.dma_start(khi[:, :n_dt // 2], gkh1[rr])
                    nc.sync.dma_start(khi[:, n_dt // 2:], gkh2[rr])
                    nc.sync.dma_start(klo[:, :n_dt // 2], gkl1[rr])
                    nc.sync.dma_start(klo[:, n_dt // 2:], gkl2[rr])
                    for it in range(n_it):
                        isl = slice(it * P, (it + 1) * P)
                        ps = spsum.tile([P, sh], f32, tag="sps", name="sps")
                        n_acc = 3 * n_dt
                        i_acc = 0
                        # khi-gated terms first so matmuls can start before
                        # the klo all-gather lands
                        for dtt in range(n_dt):
                            for lhsT, rhs in (
                                (qthi[:, dtt, isl], khi[:, dtt]),
                                (qtlo[:, dtt, isl], khi[:, dtt]),
                            ):
                                nc.tensor.matmul(
                                    ps[:], lhsT, rhs,
                                    start=(i_acc == 0), stop=(i_acc == n_acc - 1),
                                )
                                i_acc += 1
                        for dtt in range(n_dt):
                            nc.tensor.matmul(
                                ps[:], qthi[:, dtt, isl], klo[:, dtt],
                                start=(i_acc == 0), stop=(i_acc == n_acc - 1),
                            )
                            i_acc += 1
                        if rr == 0:
                            nc.vector.reduce_max(
                                m_t[it][:], ps[:], axis=mybir.AxisListType.X
                            )
                        else:
                            nc.vector.reduce_max(
                                tmpmax[:], ps[:], axis=mybir.AxisListType.X
                            )
                            nc.vector.tensor_max(m_t[it][:], m_t[it][:], tmpmax[:])
                        nc.scalar.copy(
                            s_sb[it][:, rr * sh:(rr + 1) * sh], ps[:]
                        )

                # V all-gather issued after the S-phase chunk loads so their
                # collective-tick waits don't include it (gpsimd runs it as
                # soon as bounce_v is written, long before PV needs it)
                nc.gpsimd.collective_compute(
                    "AllGather", mybir.AluOpType.bypass, replica_groups=rg,
                    ins=[bounce_v.ap().opt()], outs=[gath_v.ap().opt()],
                )

                # ---- softmax: P = exp((S - m)/sqrt(d)), ell = row sums ----
                p_sb = [prow.tile([P, nq], bf16, tag="p", name="p") for _ in range(n_it)]
                for it in range(n_it):
                    nc.vector.tensor_scalar_mul(bias_t[it][:], m_t[it][:], -scale)
                    nc.scalar.activation(
                        p_sb[it][:], s_sb[it][:],
                        mybir.ActivationFunctionType.Exp,
                        bias=bias_t[it][:], scale=scale,
                        accum_out=ell_t[it][:],
                    )
                    nc.vector.reciprocal(rl_t[it][:], ell_t[it][:])

                _spsum_cm.__exit__(None, None, None)
                # ---- O = (P @ V) / ell, two i-tile groups ----
                _ptpsum_cm = tc.tile_pool(name="ptpsum", bufs=2, space="PSUM")
                ptpsum = _ptpsum_cm.__enter__()
                _pvpsum_cm = tc.tile_pool(
                    name="pvpsum", bufs=IT_GROUP * n_eh, space="PSUM"
                )
                pvpsum = _pvpsum_cm.__enter__()
                for g in range(0, n_it, IT_GROUP):
                    its = range(g, min(g + IT_GROUP, n_it))
                    pts = {}
                    for it in its:
                        pt = ptp.tile([P, n_jt, P], bf16, tag="pt", name="pt")
                        for jt in range(n_jt):
                            tps = ptpsum.tile([P, P], bf16, tag="ptps", name="ptps")
                            nc.tensor.transpose(
                                tps[:], p_sb[it][:, jt * P:(jt + 1) * P], ident_b
                            )
                            nc.vector.tensor_copy(pt[:, jt], tps[:])
                        pts[it] = pt

                    pso = {
                        (it, eh): pvpsum.tile([P, EH], f32, tag="pvps", name="pvps")
                        for it in its for eh in range(n_eh)
                    }
                    for jg in range(n_jg):
                        vc = vchunk.tile([P, JG, dim], bf16, tag="vc", name="vc")
                        nc.sync.dma_start(vc[:], gv[jg])
                        for it in its:
                            for eh in range(n_eh):
                                esl = slice(eh * EH, (eh + 1) * EH)
                                for jj in range(JG):
                                    nc.tensor.matmul(
                                        pso[(it, eh)][:],
                                        pts[it][:, jg * JG + jj],
                                        vc[:, jj, esl],
                                        start=(jg == 0 and jj == 0),
                                        stop=(jg == n_jg - 1 and jj == JG - 1),
                                    )
                    for it in its:
                        o_sb = opool.tile([P, dim], f32, tag="o", name="o")
                        for eh in range(n_eh):
                            esl = slice(eh * EH, (eh + 1) * EH)
                            nc.vector.tensor_scalar_mul(
                                o_sb[:, esl], pso[(it, eh)][:], rl_t[it][:]
                            )
                        nc.sync.dma_start(out_ext[it * P:(it + 1) * P, :], o_sb[:])
                _pvpsum_cm.__exit__(None, None, None) if it == n_it - 1 else None
                _ptpsum_cm.__exit__(None, None, None) if it == n_it - 1 else None

    return nc


_CACHE = {}
RUN_KW = {}


def _get_nc():
    if "nc" not in _CACHE:
        _CACHE["nc"] = build_attention()
    return _CACHE["nc"]


def kernel(**inputs):
    from concourse.bass_utils import run_bass_kernel_spmd

    q = np.ascontiguousarray(np.asarray(inputs["q"], dtype=np.float32))
    k = np.ascontiguousarray(np.asarray(inputs["k"], dtype=np.float32))
    v = np.ascontiguousarray(np.asarray(inputs["v"], dtype=np.float32))
    W_q = np.ascontiguousarray(np.asarray(inputs["W_q"], dtype=np.float32))
    W_k = np.ascontiguousarray(np.asarray(inputs["W_k"], dtype=np.float32))
    W_v = np.ascontiguousarray(np.asarray(inputs["W_v"], dtype=np.float32))

    sh = N_Q // CORES
    in_maps = []
    for r in range(CORES):
        sl = slice(r * sh, (r + 1) * sh)
        in_maps.append({
            "q": q[sl], "k": k[sl], "v": v[sl],
            "W_q": W_q, "W_k": W_k, "W_v": W_v,
        })

    nc = _get_nc()
    if not nc.is_finalized():
        nc.finalize()
    res = run_bass_kernel_spmd(nc, in_maps, core_ids=list(range(CORES)), **RUN_KW)
    _CACHE["last_result"] = res
    out = np.concatenate([res.results[r]["out"] for r in range(CORES)], axis=0)
    return out


if __name__ == "__main__":
    import reference

    inputs = {kk: np.asarray(vv) for kk, vv in reference.setup_inputs().items()}
    out = kernel(**inputs)
    print("out shape:", out.shape, out.dtype)



# revision 8
# speedup vs baseline: 1.6754x; 1.6754x over previous
"""Distributed attention layer kernel for 8 TRN2 NeuronCores.

Reference computation (f32):
    Q = q @ W_q; K = k @ W_k; V = v @ W_v
    out = softmax((Q @ K^T)/sqrt(d_k)) @ V

Sharding: rows of q/k/v are split 8 ways (sequence parallel). Each core
projects its own shards, the K^T/V projections are all-gathered (fp16),
and each core computes its 512-row slice of the attention output.

Precision: projections run in f32r (fp32 operands, PE rounds mantissas
to 11 bits, full rate for free-dim >= 256) with f32 PSUM accumulation.
K^T/Q^T/V are downcast to fp16 for the attention matmuls (QK^T and PV
single plain fp16 matmuls, f32 accumulation). Softmax is f32 (ACT exp
with per-row max bias, fused row-sum). Measured end-to-end error vs the
f32 reference: ~8e-3 (gate 2e-2).
"""

import os
import sys

for _p in ("/opt/pypackages", "/opt/trn_rl_repo"):
    if _p not in sys.path:
        sys.path.insert(0, _p)

import numpy as np

N_Q, N_KV, DIM = 4096, 4096, 1024  # D_K = D_V = DIM (square weights)
CORES = 8

P = 128


def build_attention(nq=N_Q, dim=DIM, cores=CORES):
    """Build the per-core Bass graph (SPMD; identical on all cores)."""
    import concourse.bass as bass
    import concourse.mybir as mybir
    from concourse import bacc
    from concourse.masks import make_identity
    from concourse.tile import TileContext

    dt = mybir.dt
    f32, f32r, f16 = dt.float32, dt.float32r, dt.float16

    sh = nq // cores          # rows per core (512)
    n_ct = dim // P           # contraction tiles for projections (8)
    n_dt = dim // P           # d tiles (8)
    n_it = sh // P            # query-row tiles per core (4)
    n_jjt = sh // P           # kv-row tiles per core (4)
    n_jt = nq // P            # total kv j tiles (32)
    JG = 4                    # j-tiles per PV V-chunk
    n_jg = n_jt // JG         # V chunk count (8)
    EH = 512
    n_eh = dim // EH          # 512-wide output column halves (2)
    hd = dim // 2
    nh = n_dt // 2
    scale = 1.0 / float(np.sqrt(dim))

    nc = bacc.Bacc(num_devices=cores)

    # --- external I/O (per core: row shards of q/k/v, full weights) ---
    q_ext = nc.declare_dram_parameter("q", [sh, dim], f32, isOutput=False)
    k_ext = nc.declare_dram_parameter("k", [sh, dim], f32, isOutput=False)
    v_ext = nc.declare_dram_parameter("v", [sh, dim], f32, isOutput=False)
    wq_ext = nc.declare_dram_parameter("W_q", [dim, dim], f32r, isOutput=False)
    wk_ext = nc.declare_dram_parameter("W_k", [dim, dim], f32r, isOutput=False)
    wv_ext = nc.declare_dram_parameter("W_v", [dim, dim], f32r, isOutput=False)
    out_ext = nc.declare_dram_parameter("out", [sh, dim], f32, isOutput=True)

    # --- internal DRAM for collectives ---
    bounce_k1 = nc.dram_tensor("bounce_k1", [hd, sh], f16)
    bounce_k2 = nc.dram_tensor("bounce_k2", [hd, sh], f16)
    bounce_v = nc.dram_tensor("bounce_v", [sh, dim], f16)
    gath_k1 = nc.dram_tensor("gath_k1", [cores * hd, sh], f16, addr_space="Shared")
    gath_k2 = nc.dram_tensor("gath_k2", [cores * hd, sh], f16, addr_space="Shared")
    gath_v = nc.dram_tensor("gath_v", [cores * sh, dim], f16, addr_space="Shared")

    rg = [list(range(cores))]

    with TileContext(nc) as tc:
        with (
            tc.tile_pool(name="const", bufs=1) as constp,
            tc.tile_pool(name="qt", bufs=1) as qtp,
            tc.tile_pool(name="stats", bufs=1) as statp,
        ):
            # NOTE: make_identity/PE-transpose on float32r crashes walrus
            # codegen; transposes run in plain f32 and the psum result is
            # copy-cast (bit-identical) into float32r SBUF tiles.
            ident_f = constp.tile([P, P], f32, tag="idf", name="idf")
            make_identity(nc, ident_f)

            qthi = qtp.tile([P, n_dt, sh], f16, tag="qthi", name="qthi")

            with (
                tc.tile_pool(name="w", bufs=1) as wpool,
                tc.tile_pool(name="iost", bufs=6) as iost,
                tc.tile_pool(name="tin", bufs=2) as tpool,
                tc.tile_pool(name="kvout", bufs=1) as kvout,
                tc.tile_pool(name="tpsum", bufs=4, space="PSUM") as tpsum,
                tc.tile_pool(name="ppsum", bufs=2, space="PSUM") as ppsum,
            ):
                # Inputs stream on the sync (SP) HWDGE queue; weights, bounce
                # and output traffic ride the Activation HWDGE queue so the
                # two never serialize behind each other.
                def load_input(x_ext):
                    stgs = []
                    xsrc = x_ext.rearrange("(it p) c -> p it c", p=P)
                    for it in range(sh // P):
                        stg = iost.tile([P, dim], f32, tag="iostg", name="iostg")
                        nc.sync.dma_start(stg[:], xsrc[:, it])
                        stgs.append(stg)
                    return stgs

                k_stg = load_input(k_ext)

                wk = wpool.tile([P, n_ct, dim], f32r, tag="wk", name="wk")
                wq = wpool.tile([P, n_ct, dim], f32r, tag="wq", name="wq")
                wv = wpool.tile([P, n_ct, dim], f32r, tag="wv", name="wv")
                wk_src = wk_ext.rearrange("(ct p) d -> p ct d", p=P)
                wq_src = wq_ext.rearrange("(ct p) d -> p ct d", p=P)
                wv_src = wv_ext.rearrange("(ct p) d -> p ct d", p=P)
                nc.scalar.dma_start(wk[:, :, :hd], wk_src[:, :, :hd])
                nc.scalar.dma_start(wk[:, :, hd:], wk_src[:, :, hd:])

                q_stg = load_input(q_ext)

                nc.scalar.dma_start(wq[:, :, :hd], wq_src[:, :, :hd])
                nc.scalar.dma_start(wq[:, :, hd:], wq_src[:, :, hd:])
                nc.scalar.dma_start(wv[:, :, :hd], wv_src[:, :, :hd])
                nc.scalar.dma_start(wv[:, :, hd:], wv_src[:, :, hd:])

                def transpose_input(stgs, tag):
                    """Transpose a staged [sh, dim] f32 input on the PE into a
                    [c_in=128, ct, row] f32r SBUF tile (copy-cast from psum)."""
                    xt = tpool.tile([P, n_ct, sh], f32r, tag=tag, name=tag)
                    for it, stg in enumerate(stgs):
                        dst = slice(it * P, (it + 1) * P)
                        for ct in range(n_ct):
                            ps = tpsum.tile([P, P], f32, tag="tps", name="tps")
                            nc.tensor.transpose(
                                ps[:], stg[:, ct * P:(ct + 1) * P], ident_f
                            )
                            nc.vector.tensor_copy(xt[:, ct, dst], ps[:])
                    return xt

                # ---- K path first: project K^T, bounce out, all-gather.
                # The projection runs in two d-halves so the first gather
                # starts while the second half is still on the PE. ----
                kt = transpose_input(k_stg, "xt")
                kt_loc = kvout.tile([P, n_dt, sh], f16, tag="kt_loc", name="kt_loc")
                bk1 = bounce_k1.rearrange("(dtt p) jj -> p dtt jj", p=P)
                bk2 = bounce_k2.rearrange("(dtt p) jj -> p dtt jj", p=P)
                for half, (dlo, dhi, bk, bounce, gath) in enumerate((
                    (0, nh, bk1, bounce_k1, gath_k1),
                    (nh, n_dt, bk2, bounce_k2, gath_k2),
                )):
                    for dtt in range(dlo, dhi):
                        ps = ppsum.tile([P, sh], f32, tag="pps", name="pps")
                        dsl = slice(dtt * P, (dtt + 1) * P)
                        for ct in range(n_ct):
                            nc.tensor.matmul(
                                ps[:], wk[:, ct, dsl], kt[:, ct],
                                start=(ct == 0), stop=(ct == n_ct - 1),
                            )
                        nc.scalar.copy(kt_loc[:, dtt], ps[:])
                    nc.scalar.dma_start(bk[:], kt_loc[:, dlo:dhi])
                    nc.gpsimd.collective_compute(
                        "AllGather", mybir.AluOpType.bypass, replica_groups=rg,
                        ins=[bounce.ap().opt()], outs=[gath.ap().opt()],
                    )

                # ---- Q path (local only): project Q^T, downcast to fp16 ----
                qt = transpose_input(q_stg, "xt")
                for dtt in range(n_dt):
                    ps = ppsum.tile([P, sh], f32, tag="pps", name="pps")
                    dsl = slice(dtt * P, (dtt + 1) * P)
                    for ct in range(n_ct):
                        nc.tensor.matmul(
                            ps[:], wq[:, ct, dsl], qt[:, ct],
                            start=(ct == 0), stop=(ct == n_ct - 1),
                        )
                    nc.scalar.copy(qthi[:, dtt], ps[:])

                # ---- V path: project V shard, downcast, bounce ----
                v_stg = load_input(v_ext)
                vt = transpose_input(v_stg, "xt")
                v_loc = kvout.tile([P, n_jjt, dim], f16, tag="v_loc", name="v_loc")
                for jjt in range(n_jjt):
                    jsl = slice(jjt * P, (jjt + 1) * P)
                    for eh in range(n_eh):
                        ps = ppsum.tile([P, EH], f32, tag="ppsv", name="ppsv")
                        esl = slice(eh * EH, (eh + 1) * EH)
                        for ct in range(n_ct):
                            nc.tensor.matmul(
                                ps[:], vt[:, ct, jsl], wv[:, ct, esl],
                                start=(ct == 0), stop=(ct == n_ct - 1),
                            )
                        nc.scalar.copy(v_loc[:, jjt, esl], ps[:])

                bv = bounce_v.rearrange("(jjt p) e -> p jjt e", p=P)
                nc.scalar.dma_start(bv[:], v_loc[:])

            # ================= attention phase =================
            m_t = [statp.tile([P, 1], f32, tag=f"m{it}", name=f"m{it}") for it in range(n_it)]
            tmpmax = statp.tile([P, 1], f32, tag="tmpmax", name="tmpmax")
            bias_t = [statp.tile([P, 1], f32, tag=f"b{it}", name=f"b{it}") for it in range(n_it)]
            ell_t = [statp.tile([P, 1], f32, tag=f"l{it}", name=f"l{it}") for it in range(n_it)]
            rl_t = [statp.tile([P, 1], f32, tag=f"r{it}", name=f"r{it}") for it in range(n_it)]

            gk1 = gath_k1.rearrange("(r dtt p) jj -> r p dtt jj", r=cores, p=P)
            gk2 = gath_k2.rearrange("(r dtt p) jj -> r p dtt jj", r=cores, p=P)
            gv = gath_v.rearrange("(jg jj p) e -> jg p jj e", jj=JG, p=P)

            with (
                tc.tile_pool(name="schunk", bufs=4) as schunk,
                tc.tile_pool(name="srow", bufs=n_it) as srow,
                tc.tile_pool(name="prow", bufs=n_it) as prow,
                tc.tile_pool(name="ptp", bufs=1) as ptp,
                tc.tile_pool(name="vchunk", bufs=3) as vchunk,
                tc.tile_pool(name="opool", bufs=2) as opool,
            ):
                s_sb = [srow.tile([P, nq], f32, tag="s", name="s") for _ in range(n_it)]

                # ---- scores: S[it, rr-chunk] with running row max ----
                _spsum_cm = tc.tile_pool(name="spsum", bufs=6, space="PSUM")
                spsum = _spsum_cm.__enter__()
                for rr in range(cores):
                    khi = schunk.tile([P, n_dt, sh], f16, tag="khi", name="khi")
                    nc.sync.dma_start(khi[:, :nh], gk1[rr])
                    nc.sync.dma_start(khi[:, nh:], gk2[rr])
                    for it in range(n_it):
                        isl = slice(it * P, (it + 1) * P)
                        ps = spsum.tile([P, sh], f32, tag="sps", name="sps")
                        for dtt in range(n_dt):
                            nc.tensor.matmul(
                                ps[:], qthi[:, dtt, isl], khi[:, dtt],
                                start=(dtt == 0), stop=(dtt == n_dt - 1),
                            )
                        if rr == 0:
                            nc.vector.reduce_max(
                                m_t[it][:], ps[:], axis=mybir.AxisListType.X
                            )
                        else:
                            nc.vector.reduce_max(
                                tmpmax[:], ps[:], axis=mybir.AxisListType.X
                            )
                            nc.vector.tensor_max(m_t[it][:], m_t[it][:], tmpmax[:])
                        nc.scalar.copy(
                            s_sb[it][:, rr * sh:(rr + 1) * sh], ps[:]
                        )

                # V all-gather issued after the S-phase chunk loads so their
                # collective-tick waits don't include it (gpsimd runs it as
                # soon as bounce_v is written, long before PV needs it)
                nc.gpsimd.collective_compute(
                    "AllGather", mybir.AluOpType.bypass, replica_groups=rg,
                    ins=[bounce_v.ap().opt()], outs=[gath_v.ap().opt()],
                )

                # ---- softmax: P = exp((S - m)/sqrt(d)); P^T runs on the DMA
                #      XBAR (Activation HWDGE queue), not the PE ----
                p_sb = [prow.tile([P, nq], f16, tag="p", name="p") for _ in range(n_it)]
                pt = [
                    ptp.tile([P, n_jt, P], f16, tag=f"pt{it}", name=f"pt{it}")
                    for it in range(n_it)
                ]
                for it in range(n_it):
                    nc.vector.tensor_scalar_mul(bias_t[it][:], m_t[it][:], -scale)
                    nc.scalar.activation(
                        p_sb[it][:], s_sb[it][:],
                        mybir.ActivationFunctionType.Exp,
                        bias=bias_t[it][:], scale=scale,
                        accum_out=ell_t[it][:],
                    )
                    nc.vector.reciprocal(rl_t[it][:], ell_t[it][:])
                    nc.scalar.dma_start_transpose(pt[it][:], p_sb[it][:])
                _spsum_cm.__exit__(None, None, None)

                # ---- O = (P @ V) / ell, all 8 PSUM banks, single V pass ----
                _pvpsum_cm = tc.tile_pool(name="pvpsum", bufs=n_it * n_eh, space="PSUM")
                pvpsum = _pvpsum_cm.__enter__()
                pso = {
                    (it, eh): pvpsum.tile([P, EH], f32, tag="pvps", name="pvps")
                    for it in range(n_it) for eh in range(n_eh)
                }
                for jg in range(n_jg):
                    vc = vchunk.tile([P, JG, dim], f16, tag="vc", name="vc")
                    nc.sync.dma_start(vc[:], gv[jg])
                    for it in range(n_it):
                        for eh in range(n_eh):
                            esl = slice(eh * EH, (eh + 1) * EH)
                            for jj in range(JG):
                                nc.tensor.matmul(
                                    pso[(it, eh)][:],
                                    pt[it][:, jg * JG + jj],
                                    vc[:, jj, esl],
                                    start=(jg == 0 and jj == 0),
                                    stop=(jg == n_jg - 1 and jj == JG - 1),
                                )
                for it in range(n_it):
                    o_sb = opool.tile([P, dim], f32, tag="o", name="o")
                    for eh in range(n_eh):
                        esl = slice(eh * EH, (eh + 1) * EH)
                        nc.vector.tensor_scalar_mul(
                            o_sb[:, esl], pso[(it, eh)][:], rl_t[it][:]
                        )
                    nc.scalar.dma_start(out_ext[it * P:(it + 1) * P, :], o_sb[:])
                _pvpsum_cm.__exit__(None, None, None)

    return nc


_CACHE = {}
RUN_KW = {}


def _get_nc():
    if "nc" not in _CACHE:
        _CACHE["nc"] = build_attention()
    return _CACHE["nc"]


def kernel(**inputs):
    from concourse.bass_utils import run_bass_kernel_spmd

    q = np.ascontiguousarray(np.asarray(inputs["q"], dtype=np.float32))
    k = np.ascontiguousarray(np.asarray(inputs["k"], dtype=np.float32))
    v = np.ascontiguousarray(np.asarray(inputs["v"], dtype=np.float32))
    W_q = np.ascontiguousarray(np.asarray(inputs["W_q"], dtype=np.float32))
    W_k = np.ascontiguousarray(np.asarray(inputs["W_k"], dtype=np.float32))
    W_v = np.ascontiguousarray(np.asarray(inputs["W_v"], dtype=np.float32))

    sh = N_Q // CORES
    in_maps = []
    for r in range(CORES):
        sl = slice(r * sh, (r + 1) * sh)
        in_maps.append({
            "q": q[sl], "k": k[sl], "v": v[sl],
            "W_q": W_q, "W_k": W_k, "W_v": W_v,
        })

    nc = _get_nc()
    if not nc.is_finalized():
        nc.finalize()
    res = run_bass_kernel_spmd(nc, in_maps, core_ids=list(range(CORES)), **RUN_KW)
    _CACHE["last_result"] = res
    out = np.concatenate([res.results[r]["out"] for r in range(CORES)], axis=0)
    return out


if __name__ == "__main__":
    import reference

    inputs = {kk: np.asarray(vv) for kk, vv in reference.setup_inputs().items()}
    out = kernel(**inputs)
    print("out shape:", out.shape, out.dtype)


# revision 16
# speedup vs baseline: 1.7411x; 1.0392x over previous
"""Distributed attention layer kernel for 8 TRN2 NeuronCores.

Reference computation (f32):
    Q = q @ W_q; K = k @ W_k; V = v @ W_v
    out = softmax((Q @ K^T)/sqrt(d_k)) @ V

Sharding: rows of q/k/v are split 8 ways (sequence parallel). Each core
projects its own shards, the K^T/V projections are all-gathered (fp16),
and each core computes its 512-row slice of the attention output.

Precision: projections run in f32r (fp32 operands, PE rounds mantissas
to 11 bits, full rate for free-dim >= 256) with f32 PSUM accumulation.
K^T/Q^T/V are downcast to fp16 for the attention matmuls (QK^T and PV
single plain fp16 matmuls, f32 accumulation). Softmax is f32 (ACT exp
with per-row max bias, fused row-sum). Measured end-to-end error vs the
f32 reference: ~8e-3 (gate 2e-2).
"""

import os
import sys

for _p in ("/opt/pypackages", "/opt/trn_rl_repo"):
    if _p not in sys.path:
        sys.path.insert(0, _p)

import numpy as np

N_Q, N_KV, DIM = 4096, 4096, 1024  # D_K = D_V = DIM (square weights)
CORES = 8

P = 128


def build_attention(nq=N_Q, dim=DIM, cores=CORES):
    """Build the per-core Bass graph (SPMD; identical on all cores)."""
    import concourse.bass as bass
    import concourse.mybir as mybir
    from concourse import bacc
    from concourse.masks import make_identity
    from concourse.tile import TileContext

    dt = mybir.dt
    f32, f32r, f16 = dt.float32, dt.float32r, dt.float16

    sh = nq // cores          # rows per core (512)
    n_ct = dim // P           # contraction tiles for projections (8)
    n_dt = dim // P           # d tiles (8)
    n_it = sh // P            # query-row tiles per core (4)
    n_jjt = sh // P           # kv-row tiles per core (4)
    n_jt = nq // P            # total kv j tiles (32)
    JG = 4                    # j-tiles per PV V-chunk
    n_jg = n_jt // JG         # V chunk count (8)
    EH = 512
    n_eh = dim // EH          # 512-wide output column halves (2)
    hd = dim // 2
    nh = n_dt // 2
    scale = 1.0 / float(np.sqrt(dim))

    nc = bacc.Bacc(num_devices=cores)

    # --- external I/O (per core: row shards of q/k/v, full weights) ---
    q_ext = nc.declare_dram_parameter("q", [sh, dim], f32, isOutput=False)
    k_ext = nc.declare_dram_parameter("k", [sh, dim], f32, isOutput=False)
    v_ext = nc.declare_dram_parameter("v", [sh, dim], f32, isOutput=False)
    wq_ext = nc.declare_dram_parameter("W_q", [dim, dim], f32r, isOutput=False)
    wk_ext = nc.declare_dram_parameter("W_k", [dim, dim], f32r, isOutput=False)
    wv_ext = nc.declare_dram_parameter("W_v", [dim, dim], f32r, isOutput=False)
    out_ext = nc.declare_dram_parameter("out", [sh, dim], f32, isOutput=True)

    # --- internal DRAM for collectives ---
    bounce_k = nc.dram_tensor("bounce_k", [dim, sh], f16)
    bounce_v = nc.dram_tensor("bounce_v", [sh, dim], f16)
    gath_k = nc.dram_tensor("gath_k", [cores * dim, sh], f16, addr_space="Shared")
    gath_v = nc.dram_tensor("gath_v", [cores * sh, dim], f16, addr_space="Shared")

    rg = [list(range(cores))]

    with TileContext(nc) as tc:
        with (
            tc.tile_pool(name="const", bufs=1) as constp,
            tc.tile_pool(name="qt", bufs=1) as qtp,
            tc.tile_pool(name="stats", bufs=1) as statp,
        ):
            # NOTE: make_identity/PE-transpose on float32r crashes walrus
            # codegen; transposes run in plain f32 and the psum result is
            # copy-cast (bit-identical) into float32r SBUF tiles.
            ident_f = constp.tile([P, P], f32, tag="idf", name="idf")
            make_identity(nc, ident_f)

            qthi = qtp.tile([P, n_dt, sh], f16, tag="qthi", name="qthi")

            with (
                tc.tile_pool(name="w", bufs=1) as wpool,
                tc.tile_pool(name="iost", bufs=6) as iost,
                tc.tile_pool(name="tin", bufs=2) as tpool,
                tc.tile_pool(name="kvout", bufs=1) as kvout,
                tc.tile_pool(name="tpsum", bufs=4, space="PSUM") as tpsum,
                tc.tile_pool(name="ppsum", bufs=2, space="PSUM") as ppsum,
            ):
                # All bulk loads (inputs + weights) stream in order on the
                # sync (SP) HWDGE queue; the Activation HWDGE queue is kept
                # for small latency-critical transfers (bounce buffers, P^T
                # XBAR transposes, outputs) so their triggers never stall the
                # ACT engine behind megabytes of weight traffic.
                def load_input(x_ext):
                    stgs = []
                    xsrc = x_ext.rearrange("(it p) c -> p it c", p=P)
                    for it in range(sh // P):
                        stg = iost.tile([P, dim], f32, tag="iostg", name="iostg")
                        nc.sync.dma_start(stg[:], xsrc[:, it])
                        stgs.append(stg)
                    return stgs

                wk = wpool.tile([P, n_ct, dim], f32r, tag="wk", name="wk")
                wq = wpool.tile([P, n_ct, dim], f32r, tag="wq", name="wq")
                wv = wpool.tile([P, n_ct, dim], f32r, tag="wv", name="wv")
                wk_src = wk_ext.rearrange("(ct p) d -> p ct d", p=P)
                wq_src = wq_ext.rearrange("(ct p) d -> p ct d", p=P)
                wv_src = wv_ext.rearrange("(ct p) d -> p ct d", p=P)

                k_stg = load_input(k_ext)
                nc.sync.dma_start(wk[:, :, :hd], wk_src[:, :, :hd])
                nc.sync.dma_start(wk[:, :, hd:], wk_src[:, :, hd:])
                q_stg = load_input(q_ext)
                nc.sync.dma_start(wq[:, :, :hd], wq_src[:, :, :hd])
                nc.sync.dma_start(wq[:, :, hd:], wq_src[:, :, hd:])
                nc.sync.dma_start(wv[:, :, :hd], wv_src[:, :, :hd])
                nc.sync.dma_start(wv[:, :, hd:], wv_src[:, :, hd:])

                def transpose_input(stgs, tag):
                    """Transpose a staged [sh, dim] f32 input on the PE into a
                    [c_in=128, ct, row] f32r SBUF tile (copy-cast from psum)."""
                    xt = tpool.tile([P, n_ct, sh], f32r, tag=tag, name=tag)
                    for it, stg in enumerate(stgs):
                        dst = slice(it * P, (it + 1) * P)
                        for ct in range(n_ct):
                            ps = tpsum.tile([P, P], f32, tag="tps", name="tps")
                            nc.tensor.transpose(
                                ps[:], stg[:, ct * P:(ct + 1) * P], ident_f
                            )
                            nc.vector.tensor_copy(xt[:, ct, dst], ps[:])
                    return xt

                # ---- K path first: project K^T, bounce out, all-gather.
                # A single gather: the kernel-entry CC barrier (~45us of
                # cross-core launch skew) gates the first collective anyway,
                # so splitting the gather only adds trigger latency. ----
                kt = transpose_input(k_stg, "xt")
                kt_loc = kvout.tile([P, n_dt, sh], f16, tag="kt_loc", name="kt_loc")
                bk = bounce_k.rearrange("(dtt p) jj -> p dtt jj", p=P)
                for dtt in range(n_dt):
                    ps = ppsum.tile([P, sh], f32, tag="pps", name="pps")
                    dsl = slice(dtt * P, (dtt + 1) * P)
                    for ct in range(n_ct):
                        nc.tensor.matmul(
                            ps[:], wk[:, ct, dsl], kt[:, ct],
                            start=(ct == 0), stop=(ct == n_ct - 1),
                        )
                    nc.scalar.copy(kt_loc[:, dtt], ps[:])
                nc.scalar.dma_start(bk[:], kt_loc[:])
                nc.gpsimd.collective_compute(
                    "AllGather", mybir.AluOpType.bypass, replica_groups=rg,
                    ins=[bounce_k.ap().opt()], outs=[gath_k.ap().opt()],
                )

                # ---- Q path (local only): project Q^T, downcast to fp16 ----
                qt = transpose_input(q_stg, "xt")
                for dtt in range(n_dt):
                    ps = ppsum.tile([P, sh], f32, tag="pps", name="pps")
                    dsl = slice(dtt * P, (dtt + 1) * P)
                    for ct in range(n_ct):
                        nc.tensor.matmul(
                            ps[:], wq[:, ct, dsl], qt[:, ct],
                            start=(ct == 0), stop=(ct == n_ct - 1),
                        )
                    nc.scalar.copy(qthi[:, dtt], ps[:])

                # ---- V path: project V shard, downcast, bounce ----
                v_stg = load_input(v_ext)
                vt = transpose_input(v_stg, "xt")
                v_loc = kvout.tile([P, n_jjt, dim], f16, tag="v_loc", name="v_loc")
                for jjt in range(n_jjt):
                    jsl = slice(jjt * P, (jjt + 1) * P)
                    for eh in range(n_eh):
                        ps = ppsum.tile([P, EH], f32, tag="ppsv", name="ppsv")
                        esl = slice(eh * EH, (eh + 1) * EH)
                        for ct in range(n_ct):
                            nc.tensor.matmul(
                                ps[:], vt[:, ct, jsl], wv[:, ct, esl],
                                start=(ct == 0), stop=(ct == n_ct - 1),
                            )
                        nc.scalar.copy(v_loc[:, jjt, esl], ps[:])

                bv = bounce_v.rearrange("(jjt p) e -> p jjt e", p=P)
                nc.scalar.dma_start(bv[:], v_loc[:])

            # ================= attention phase =================
            m_t = [statp.tile([P, 1], f32, tag=f"m{it}", name=f"m{it}") for it in range(n_it)]
            tmpmax = statp.tile([P, 1], f32, tag="tmpmax", name="tmpmax")
            bias_t = [statp.tile([P, 1], f32, tag=f"b{it}", name=f"b{it}") for it in range(n_it)]
            ell_t = [statp.tile([P, 1], f32, tag=f"l{it}", name=f"l{it}") for it in range(n_it)]
            rl_t = [statp.tile([P, 1], f32, tag=f"r{it}", name=f"r{it}") for it in range(n_it)]

            gk = gath_k.rearrange("(r dtt p) jj -> r p dtt jj", r=cores, p=P)
            gv = gath_v.rearrange("(jg jj p) e -> jg p jj e", jj=JG, p=P)

            with (
                tc.tile_pool(name="schunk", bufs=4) as schunk,
                tc.tile_pool(name="srow", bufs=n_it) as srow,
                tc.tile_pool(name="prow", bufs=n_it) as prow,
                tc.tile_pool(name="ptp", bufs=1) as ptp,
                tc.tile_pool(name="vchunk", bufs=3) as vchunk,
                tc.tile_pool(name="opool", bufs=2) as opool,
            ):
                s_sb = [srow.tile([P, nq], f32, tag="s", name="s") for _ in range(n_it)]

                # ---- scores: S[it, rr-chunk] with running row max ----
                _spsum_cm = tc.tile_pool(name="spsum", bufs=6, space="PSUM")
                spsum = _spsum_cm.__enter__()
                for rr in range(cores):
                    khi = schunk.tile([P, n_dt, sh], f16, tag="khi", name="khi")
                    nc.sync.dma_start(khi[:], gk[rr])
                    for it in range(n_it):
                        isl = slice(it * P, (it + 1) * P)
                        ps = spsum.tile([P, sh], f32, tag="sps", name="sps")
                        for dtt in range(n_dt):
                            nc.tensor.matmul(
                                ps[:], qthi[:, dtt, isl], khi[:, dtt],
                                start=(dtt == 0), stop=(dtt == n_dt - 1),
                            )
                        if rr == 0:
                            nc.vector.reduce_max(
                                m_t[it][:], ps[:], axis=mybir.AxisListType.X
                            )
                        else:
                            nc.vector.reduce_max(
                                tmpmax[:], ps[:], axis=mybir.AxisListType.X
                            )
                            nc.vector.tensor_max(m_t[it][:], m_t[it][:], tmpmax[:])
                        nc.scalar.copy(
                            s_sb[it][:, rr * sh:(rr + 1) * sh], ps[:]
                        )

                # V all-gather issued after the S-phase chunk loads so their
                # collective-tick waits don't include it (gpsimd runs it as
                # soon as bounce_v is written, long before PV needs it)
                nc.gpsimd.collective_compute(
                    "AllGather", mybir.AluOpType.bypass, replica_groups=rg,
                    ins=[bounce_v.ap().opt()], outs=[gath_v.ap().opt()],
                )

                # ---- softmax: P = exp((S - m)/sqrt(d)); P^T runs on the DMA
                #      XBAR (Activation HWDGE queue), not the PE ----
                p_sb = [prow.tile([P, nq], f16, tag="p", name="p") for _ in range(n_it)]
                pt = [
                    ptp.tile([P, n_jt, P], f16, tag=f"pt{it}", name=f"pt{it}")
                    for it in range(n_it)
                ]
                for it in range(n_it):
                    nc.vector.tensor_scalar_mul(bias_t[it][:], m_t[it][:], -scale)
                    nc.scalar.activation(
                        p_sb[it][:], s_sb[it][:],
                        mybir.ActivationFunctionType.Exp,
                        bias=bias_t[it][:], scale=scale,
                        accum_out=ell_t[it][:],
                    )
                    nc.vector.reciprocal(rl_t[it][:], ell_t[it][:])
                    nc.scalar.dma_start_transpose(pt[it][:], p_sb[it][:])
                _spsum_cm.__exit__(None, None, None)

                # ---- O = (P @ V) / ell, all 8 PSUM banks, single V pass ----
                _pvpsum_cm = tc.tile_pool(name="pvpsum", bufs=n_it * n_eh, space="PSUM")
                pvpsum = _pvpsum_cm.__enter__()
                pso = {
                    (it, eh): pvpsum.tile([P, EH], f32, tag="pvps", name="pvps")
                    for it in range(n_it) for eh in range(n_eh)
                }
                for jg in range(n_jg):
                    vc = vchunk.tile([P, JG, dim], f16, tag="vc", name="vc")
                    nc.sync.dma_start(vc[:], gv[jg])
                    last = jg == n_jg - 1
                    for it in range(n_it):
                        for eh in range(n_eh):
                            esl = slice(eh * EH, (eh + 1) * EH)
                            for jj in range(JG):
                                nc.tensor.matmul(
                                    pso[(it, eh)][:],
                                    pt[it][:, jg * JG + jj],
                                    vc[:, jj, esl],
                                    start=(jg == 0 and jj == 0),
                                    stop=(last and jj == JG - 1),
                                )
                        if last:
                            # scale + store this row tile while the PE is
                            # still accumulating the remaining row tiles
                            o_sb = opool.tile([P, dim], f32, tag="o", name="o")
                            for eh in range(n_eh):
                                esl = slice(eh * EH, (eh + 1) * EH)
                                nc.vector.tensor_scalar_mul(
                                    o_sb[:, esl], pso[(it, eh)][:], rl_t[it][:]
                                )
                            nc.scalar.dma_start(
                                out_ext[it * P:(it + 1) * P, :], o_sb[:]
                            )
                _pvpsum_cm.__exit__(None, None, None)

    return nc


_CACHE = {}
RUN_KW = {}


def _get_nc():
    if "nc" not in _CACHE:
        _CACHE["nc"] = build_attention()
    return _CACHE["nc"]


def kernel(**inputs):
    from concourse.bass_utils import run_bass_kernel_spmd

    q = np.ascontiguousarray(np.asarray(inputs["q"], dtype=np.float32))
    k = np.ascontiguousarray(np.asarray(inputs["k"], dtype=np.float32))
    v = np.ascontiguousarray(np.asarray(inputs["v"], dtype=np.float32))
    W_q = np.ascontiguousarray(np.asarray(inputs["W_q"], dtype=np.float32))
    W_k = np.ascontiguousarray(np.asarray(inputs["W_k"], dtype=np.float32))
    W_v = np.ascontiguousarray(np.asarray(inputs["W_v"], dtype=np.float32))

    sh = N_Q // CORES
    in_maps = []
    for r in range(CORES):
        sl = slice(r * sh, (r + 1) * sh)
        in_maps.append({
            "q": q[sl], "k": k[sl], "v": v[sl],
            "W_q": W_q, "W_k": W_k, "W_v": W_v,
        })

    nc = _get_nc()
    if not nc.is_finalized():
        nc.finalize()
    res = run_bass_kernel_spmd(nc, in_maps, core_ids=list(range(CORES)), **RUN_KW)
    _CACHE["last_result"] = res
    out = np.concatenate([res.results[r]["out"] for r in range(CORES)], axis=0)
    return out


if __name__ == "__main__":
    import reference

    inputs = {kk: np.asarray(vv) for kk, vv in reference.setup_inputs().items()}
    out = kernel(**inputs)
    print("out shape:", out.shape, out.dtype)
